# revision 32
# baseline (speedup 1.0000x reference)
"""MoE expert-parallel kernel for Trainium2 (8 NeuronCores).

Strategy (expert-parallel, host-side dispatch):
  - Host sorts the T=4096 tokens by dispatch_order. Core e receives the
    tokens routed to expert e, padded to a common capacity Cap, already
    transposed to feature-major xT [D, Cap] (so tokens are always the
    matmul moving/free dimension on device; both weight matrices are used
    in their native layout as the stationary operand).
  - Device (per core): h = gelu_tanh(W1.T-block @ xT + b1) computed
    feature-major [DFF, Cap] in SBUF, then yT = W2-block @ h + b2,
    DMA'd back as [D, Cap]. Matmuls run as float32r (FP22 mantissa
    truncation, full PE rate for free-dim >= 256).
  - Host scatters yT columns back to the original token order.

Self-contained: hardcodes all shapes from the problem spec.
"""

import os
import sys
from contextlib import ExitStack

import numpy as np

for _p in ("/opt/trn_rl_repo",):
    if _p not in sys.path:
        sys.path.insert(0, _p)

import concourse.bass as bass  # noqa: E402
import concourse.tile as tile  # noqa: E402
from concourse import mybir  # noqa: E402
from concourse.bass_utils import run_bass_kernel_spmd  # noqa: E402

# ---------------------------------------------------------------------------
# Workaround for this walrus build: a Drain instruction with >1 sem wait
# fails codegen ("Too many sync wait commands").  Replace the Tile
# kernel-tail drain with single-wait SP nops followed by a bare drain.
# ---------------------------------------------------------------------------


def _patched_drain_and_barrier(self, tick_clock, wait_clock):
    from concourse.vector_clock import ScopedClock

    nc = self.nc
    probe = nc.sync.nop(nofuse=True)
    wait_clock.add_sem_waits(probe.ins, ScopedClock({None: tick_clock.global_clock}))
    si = probe.ins.sync_info
    waits = list(si.on_wait) if si and si.on_wait else []
    probe.ins.sync_info = mybir.SyncInfo(on_wait=waits[:1], on_update=[])
    for w in waits[1:]:
        n = nc.sync.nop(nofuse=True)
        n.ins.sync_info = mybir.SyncInfo(on_wait=[w], on_update=[])

    nc.sync.drain()
    nc.all_engine_barrier()
    assert self.sems is not None
    popped = nc._tile_sem_poison_stack.pop()
    assert popped is self._sem_poison
    nc.clear_and_free_semaphores(list(self.sems.allocated().values()))
    nc.all_engine_barrier()


tile.TileContext._drain_and_barrier = _patched_drain_and_barrier


def _split_excess_sync_waits(nc, max_waits=1):
    """This walrus build only encodes one sem wait per instruction.  Hoist
    excess waits onto same-engine nops inserted immediately before."""
    for f in nc.m.functions:
        for bb in f.blocks:
            out = []
            for inst in bb.instructions:
                si = inst.sync_info
                if si and si.on_wait and len(si.on_wait) > max_waits:
                    waits = list(si.on_wait)
                    for i in range(max_waits, len(waits), max_waits):
                        n = mybir.InstNoOp(
                            name=f"{inst.name}-waitsplit-{i}", ins=[], outs=[]
                        )
                        n.engine = inst.engine
                        n.sync_info = mybir.SyncInfo(
                            on_wait=waits[i : i + max_waits], on_update=[]
                        )
                        out.append(n)
                    inst.sync_info = mybir.SyncInfo(
                        on_wait=waits[:max_waits], on_update=list(si.on_update or [])
                    )
                out.append(inst)
            bb.instructions[:] = out


# ---------------------------------------------------------------------------

NUM_EXPERTS = 8
D = 1024
DFF = 4096
N_CORES = 8
KD = D // 128  # 8 k-chunks for matmul 1
FC = DFF // 128  # 32 f-chunks
DM = D // 128  # 8 output chunks
FG = 4  # f-groups for w1 streaming (each 8 f-chunks = 1024 cols)

F32 = mybir.dt.float32
F32R = mybir.dt.float32r
F16 = mybir.dt.float16

LAST_EXEC_NS = None
LAST_RESULT = None

_NC_CACHE = {}


def _plan(max_count):
    """Pick capacity/chunking: equal token chunks, each in [256, 512].

    Chunks >= ~256 keep the per-matmul LDWEIGHTS (~100 ns) hidden behind
    the matmul stream (chunk/2.4GHz ns)."""
    n_chunks = max(1, -(-max_count // 512))
    chunk = -(-max_count // n_chunks)
    chunk = max(256, -(-chunk // 8) * 8)
    cap = chunk * n_chunks
    # Tokens are processed in blocks of <=2 chunks so h/PSUM stay bounded
    # for arbitrarily skewed dispatch.  Phase 2: each pass uses
    # dm_per_pass*block_chunks PSUM banks (<=4), so two pass-sets alternate
    # within the 8 banks and passes overlap.
    dm_per_pass = max(1, 4 // min(n_chunks, 2))
    n_pass = -(-DM // dm_per_pass)
    return cap, chunk, n_chunks, dm_per_pass, n_pass


def _build_nc(cap, chunk, n_chunks, dm_per_pass, n_pass):
    nc = bass.Bass()
    xT = nc.declare_dram_parameter("xT", [D, cap], F16, isOutput=False)
    # w1 is host-packed fg-major: row fg*128+p holds that partition's eight
    # k-tiles of f-group fg contiguously, so one DMA loads a whole f-group
    # with 16KB lines (128 descriptors for 2MB).
    w1 = nc.declare_dram_parameter("w1", [FG * 128, KD * 1024], F16, isOutput=False)
    w2 = nc.declare_dram_parameter("w2", [DFF, D], F16, isOutput=False)
    b1 = nc.declare_dram_parameter("b1", [128, FC], F32, isOutput=False)
    b2 = nc.declare_dram_parameter("b2", [128, DM], F32, isOutput=False)
    yT = nc.declare_dram_parameter("yT", [D, cap], F32, isOutput=True)

    gelu = mybir.ActivationFunctionType.Gelu_apprx_tanh

    with ExitStack() as ctx:
        tc = ctx.enter_context(tile.TileContext(nc))
        bpool = ctx.enter_context(tc.tile_pool(name="bias", bufs=1))
        xpool = ctx.enter_context(tc.tile_pool(name="xT", bufs=KD))
        hpool = ctx.enter_context(tc.tile_pool(name="h", bufs=FC))
        w1pool = ctx.enter_context(tc.tile_pool(name="w1", bufs=8))
        wbig_bufs = 2 if cap <= 1024 else 1
        w2pool = ctx.enter_context(tc.tile_pool(name="w2", bufs=4))
        ypool = ctx.enter_context(tc.tile_pool(name="y", bufs=4))

        # First wave: fg0's w1 tiles interleaved with the x tiles (each a
        # single contiguous [128, N] DMA -> ~0.6us trigger each on SP).
        xts = []
        w1t0 = []
        for k in range(KD):
            w = w1pool.tile([128, 1024], F16, name="w0", tag="w0")
            nc.sync.dma_start(w[:], w1[0:128, k * 1024 : (k + 1) * 1024])
            w1t0.append(w)
            xt = xpool.tile([128, cap], F16)
            nc.sync.dma_start(xt[:], xT[k * 128 : (k + 1) * 128, :])
            xts.append(xt)

        # Biases are tiny and first needed ~16us in; load via scalar HWDGE
        # to keep the SP trigger queue clear for weights.
        b1_sb = bpool.tile([128, FC], F32, tag="b1")
        nc.scalar.dma_start(b1_sb[:], b1[:, :])
        b2_sb = bpool.tile([128, DM], F32, tag="b2")
        nc.scalar.dma_start(b2_sb[:], b2[:, :])

        # Tokens are processed in blocks of <=2 chunks: h and PSUM footprints
        # stay bounded for arbitrarily skewed dispatch; weights are
        # re-streamed per block (only one block in the common case).
        FB = max(1, 8 // dm_per_pass)  # f-blocks batched per w2 DMA
        w2p = w2.rearrange("(q p) d -> q p d", p=128)
        blocks = []
        c0 = 0
        while c0 < n_chunks:
            blocks.append((c0, min(2, n_chunks - c0)))
            c0 += 2

        for bi, (cb, ncb) in enumerate(blocks):
            tok0 = cb * chunk
            bcap = ncb * chunk

            # ---- phase 1: h = gelu(x @ W1 + b1), feature-major ----
            hs = []
            with tc.tile_pool(name=f"p1_{bi}", bufs=3, space="PSUM") as p1pool:
                for fg in range(FG):
                    if bi == 0 and fg == 0:
                        wbig = None
                    else:
                        wbig = w1pool.tile(
                            [128, KD * 1024],
                            F16,
                            name="wbig",
                            tag="wbig",
                            bufs=wbig_bufs,
                        )
                        nc.sync.dma_start(wbig[:], w1[fg * 128 : (fg + 1) * 128, :])

                    for fl in range(8):
                        f = fg * 8 + fl
                        h = hpool.tile([128, 2 * chunk], F16, name="h", tag="h")
                        for c in range(ncb):
                            t0 = tok0 + c * chunk
                            ps = p1pool.tile([128, chunk], F32, name="ps", tag="ps")
                            for k in range(KD):
                                if wbig is None:
                                    lhsT = w1t0[k][:, fl * 128 : (fl + 1) * 128]
                                else:
                                    lo = k * 1024 + fl * 128
                                    lhsT = wbig[:, lo : lo + 128]
                                nc.tensor.matmul(
                                    ps[:, :],
                                    lhsT,
                                    xts[k][:, t0 : t0 + chunk],
                                    start=(k == 0),
                                    stop=(k == KD - 1),
                                )
                            nc.scalar.activation(
                                h[:, c * chunk : (c + 1) * chunk],
                                ps[:, :],
                                gelu,
                                bias=b1_sb[:, f : f + 1],
                                scale=1.0,
                            )
                        hs.append(h)

            # ---- phase 2: yT = W2 @ h + b2 ----
            # Passes cover dm_per_pass output chunks each and alternate
            # between two PSUM bank sets so pass N+1's matmuls overlap pass
            # N's copy-out.  The host pre-packs w2 so each DMA is one
            # contiguous [128, 1024] slab; triggers go via the scalar
            # engine's HWDGE queue (idle during phase 2).
            with tc.tile_pool(name=f"p2_{bi}", bufs=1, space="PSUM") as p2pool:
                for pz in range(n_pass):
                    dm_lo = pz * dm_per_pass
                    dm_n = min(dm_per_pass, DM - dm_lo)
                    yps = {}
                    for dl in range(dm_n):
                        for c in range(ncb):
                            yps[(dl, c)] = p2pool.tile(
                                [128, chunk],
                                F32,
                                name=f"yp{pz % 2}_{dl}_{c}",
                                tag=f"yp{pz % 2}_{dl}_{c}",
                            )
                    for fq in range(FC // FB):
                        w2t = w2pool.tile([128, FB * dm_per_pass * 128], F16)
                        nc.scalar.dma_start(w2t[:], w2p[pz * (FC // FB) + fq])
                        for fb in range(FB):
                            f = fq * FB + fb
                            for dl in range(dm_n):
                                for c in range(ncb):
                                    nc.tensor.matmul(
                                        yps[(dl, c)][:, :],
                                        w2t[
                                            :,
                                            (fb * dm_n + dl) * 128 : (fb * dm_n + dl + 1) * 128,
                                        ],
                                        hs[f][:, c * chunk : (c + 1) * chunk],
                                        start=(f == 0),
                                        stop=(f == FC - 1),
                                    )
                    for dl in range(dm_n):
                        dm = dm_lo + dl
                        yt = ypool.tile([128, 2 * chunk], F32, name="yt", tag="yt")
                        for c in range(ncb):
                            nc.vector.tensor_scalar_add(
                                yt[:, c * chunk : (c + 1) * chunk],
                                yps[(dl, c)][:, :],
                                b2_sb[:, dm : dm + 1],
                            )
                        nc.sync.dma_start(
                            yT[dm * 128 : (dm + 1) * 128, tok0 : tok0 + bcap],
                            yt[:, :bcap],
                        )

    _split_excess_sync_waits(nc)
    return nc


def _pack_w1(w1e):
    """Pack one expert's w1 fg-major: row fg*128+p holds the partition's
    eight k-tiles of f-group fg contiguously (one 16KB-line DMA per fg)."""
    w = w1e.reshape(KD, 128, FG, 1024)
    packed = w.transpose(2, 1, 0, 3)  # [fg, p, k, 1024]
    return np.ascontiguousarray(packed.reshape(FG * 128, KD * 1024)).astype(
        np.float16
    )


def _pack_w2(w2e, dm_per_pass, n_pass):
    """Pre-pack one expert's w2 so each phase-2 DMA reads one contiguous
    [128, FB*dm_per_pass*128] slab in kernel consumption order."""
    FB = max(1, 8 // dm_per_pass)
    FCB = FC // FB
    w = w2e.reshape(FCB, FB, 128, n_pass, dm_per_pass, 128)
    packed = w.transpose(3, 0, 2, 1, 4, 5)  # [pz, fq, p, fb, dl, d2]
    return np.ascontiguousarray(packed.reshape(DFF, D)).astype(np.float16)


def _enable_trace_hooks():
    """Register the NTFF profile hook (missing antenv.axon_hooks shim)."""
    import types

    if "antenv.axon_hooks" not in sys.modules:
        mod = types.ModuleType("antenv.axon_hooks")
        mod._hook = None

        def set_axon_ntff_profile_hook(h):
            mod._hook = h

        def get_axon_ntff_profile_hook():
            return mod._hook

        mod.set_axon_ntff_profile_hook = set_axon_ntff_profile_hook
        mod.get_axon_ntff_profile_hook = get_axon_ntff_profile_hook
        sys.modules["antenv.axon_hooks"] = mod
        import antenv

        antenv.axon_hooks = mod
    import antenv.axon_hooks as ah

    if ah.get_axon_ntff_profile_hook() is None:
        from trn_agent_boot.trn_boot import _ntff_profile_via_ctypes

        ah.set_axon_ntff_profile_hook(
            _ntff_profile_via_ctypes("/opt/axon/libaxon_pjrt.so")
        )
    import concourse.bass_utils as bu

    bu.upload_artifacts = lambda tmpdir: "local://skipped"


def kernel(inputs, w1, b1, w2, b2, dispatch_order):
    global LAST_EXEC_NS, LAST_RESULT

    inputs = np.asarray(inputs, dtype=np.float32)
    w1 = np.asarray(w1, dtype=np.float32)
    b1 = np.asarray(b1, dtype=np.float32)
    w2 = np.asarray(w2, dtype=np.float32)
    b2 = np.asarray(b2, dtype=np.float32)
    disp = np.asarray(dispatch_order).astype(np.int64)

    B, S, _ = inputs.shape
    T = B * S
    x = inputs.reshape(T, D)

    order = np.argsort(disp, kind="stable")
    counts = np.bincount(disp, minlength=NUM_EXPERTS)
    starts = np.zeros(NUM_EXPERTS + 1, dtype=np.int64)
    np.cumsum(counts, out=starts[1:])

    cap, chunk, n_chunks, dm_per_pass, n_pass = _plan(int(counts.max()))

    key = (cap, chunk, n_chunks, dm_per_pass, n_pass)
    if key not in _NC_CACHE:
        _NC_CACHE[key] = _build_nc(*key)
    nc = _NC_CACHE[key]

    in_maps = []
    for e in range(NUM_EXPERTS):
        toks = order[starts[e] : starts[e + 1]]
        xT_e = np.zeros((D, cap), dtype=np.float16)
        if len(toks):
            xT_e[:, : len(toks)] = x[toks].T
        in_maps.append(
            {
                "xT": xT_e,
                "w1": _pack_w1(w1[e]),
                "w2": _pack_w2(w2[e], dm_per_pass, n_pass),
                "b1": np.ascontiguousarray(b1[e].reshape(FC, 128).T),
                "b2": np.ascontiguousarray(b2[e].reshape(DM, 128).T),
            }
        )

    trace = os.environ.get("MOE_TRACE") == "1"
    kwargs = {}
    if trace:
        _enable_trace_hooks()
        kwargs["trace"] = True
        tmpdir = os.environ.get("MOE_TRACE_DIR")
        if tmpdir:
            os.makedirs(tmpdir, exist_ok=True)
            kwargs["tmpdir"] = tmpdir

    res = run_bass_kernel_spmd(nc, in_maps, list(range(N_CORES)), **kwargs)
    LAST_RESULT = res
    LAST_EXEC_NS = res.exec_time_ns

    out = np.empty((T, D), dtype=np.float32)
    for e in range(NUM_EXPERTS):
        toks = order[starts[e] : starts[e + 1]]
        if len(toks):
            out[toks] = res.results[e]["yT"][:, : len(toks)].T
    return out.reshape(B, S, D)


# revision 33
# speedup vs baseline: 1.0291x; 1.0291x over previous
"""MoE expert-parallel kernel for Trainium2 (8 NeuronCores).

Strategy (expert-parallel, host-side dispatch):
  - Host sorts the T=4096 tokens by dispatch_order. Core e receives the
    tokens routed to expert e, padded to a common capacity Cap, already
    transposed to feature-major xT [D, Cap] (so tokens are always the
    matmul moving/free dimension on device; both weight matrices are used
    in their native layout as the stationary operand).
  - Device (per core): h = gelu_tanh(W1.T-block @ xT + b1) computed
    feature-major [DFF, Cap] in SBUF, then yT = W2-block @ h + b2,
    DMA'd back as [D, Cap]. Matmuls run as float32r (FP22 mantissa
    truncation, full PE rate for free-dim >= 256).
  - Host scatters yT columns back to the original token order.

Self-contained: hardcodes all shapes from the problem spec.
"""

import os
import sys
from contextlib import ExitStack

import numpy as np

for _p in ("/opt/trn_rl_repo",):
    if _p not in sys.path:
        sys.path.insert(0, _p)

import concourse.bass as bass  # noqa: E402
import concourse.tile as tile  # noqa: E402
from concourse import mybir  # noqa: E402
from concourse.bass_utils import run_bass_kernel_spmd  # noqa: E402

# ---------------------------------------------------------------------------
# Workaround for this walrus build: a Drain instruction with >1 sem wait
# fails codegen ("Too many sync wait commands").  Replace the Tile
# kernel-tail drain with single-wait SP nops followed by a bare drain.
# ---------------------------------------------------------------------------


def _patched_drain_and_barrier(self, tick_clock, wait_clock):
    from concourse.vector_clock import ScopedClock

    nc = self.nc
    probe = nc.sync.nop(nofuse=True)
    wait_clock.add_sem_waits(probe.ins, ScopedClock({None: tick_clock.global_clock}))
    si = probe.ins.sync_info
    waits = list(si.on_wait) if si and si.on_wait else []
    probe.ins.sync_info = mybir.SyncInfo(on_wait=waits[:1], on_update=[])
    for w in waits[1:]:
        n = nc.sync.nop(nofuse=True)
        n.ins.sync_info = mybir.SyncInfo(on_wait=[w], on_update=[])

    nc.sync.drain()
    nc.all_engine_barrier()
    assert self.sems is not None
    popped = nc._tile_sem_poison_stack.pop()
    assert popped is self._sem_poison
    nc.clear_and_free_semaphores(list(self.sems.allocated().values()))
    nc.all_engine_barrier()


tile.TileContext._drain_and_barrier = _patched_drain_and_barrier


def _split_excess_sync_waits(nc, max_waits=1):
    """This walrus build only encodes one sem wait per instruction.  Hoist
    excess waits onto same-engine nops inserted immediately before."""
    for f in nc.m.functions:
        for bb in f.blocks:
            out = []
            for inst in bb.instructions:
                si = inst.sync_info
                if si and si.on_wait and len(si.on_wait) > max_waits:
                    waits = list(si.on_wait)
                    for i in range(max_waits, len(waits), max_waits):
                        n = mybir.InstNoOp(
                            name=f"{inst.name}-waitsplit-{i}", ins=[], outs=[]
                        )
                        n.engine = inst.engine
                        n.sync_info = mybir.SyncInfo(
                            on_wait=waits[i : i + max_waits], on_update=[]
                        )
                        out.append(n)
                    inst.sync_info = mybir.SyncInfo(
                        on_wait=waits[:max_waits], on_update=list(si.on_update or [])
                    )
                out.append(inst)
            bb.instructions[:] = out


# ---------------------------------------------------------------------------

NUM_EXPERTS = 8
D = 1024
DFF = 4096
N_CORES = 8
KD = D // 128  # 8 k-chunks for matmul 1
FC = DFF // 128  # 32 f-chunks
DM = D // 128  # 8 output chunks
FG = 4  # f-groups for w1 streaming (each 8 f-chunks = 1024 cols)

F32 = mybir.dt.float32
F32R = mybir.dt.float32r
F16 = mybir.dt.float16

LAST_EXEC_NS = None
LAST_RESULT = None

_NC_CACHE = {}


def _plan(max_count):
    """Pick capacity/chunking: equal token chunks, each in [256, 512].

    Chunks >= ~256 keep the per-matmul LDWEIGHTS (~100 ns) hidden behind
    the matmul stream (chunk/2.4GHz ns)."""
    n_chunks = max(1, -(-max_count // 512))
    chunk = -(-max_count // n_chunks)
    chunk = max(256, -(-chunk // 8) * 8)
    cap = chunk * n_chunks
    # Tokens are processed in blocks of <=2 chunks so h/PSUM stay bounded
    # for arbitrarily skewed dispatch.  Phase 2: each pass uses
    # dm_per_pass*block_chunks PSUM banks (<=4), so two pass-sets alternate
    # within the 8 banks and passes overlap.
    dm_per_pass = max(1, 4 // min(n_chunks, 2))
    n_pass = -(-DM // dm_per_pass)
    return cap, chunk, n_chunks, dm_per_pass, n_pass


def _build_nc(cap, chunk, n_chunks, dm_per_pass, n_pass):
    nc = bass.Bass()
    xT = nc.declare_dram_parameter("xT", [D, cap], F16, isOutput=False)
    # w1 is host-packed fg-major: row fg*128+p holds that partition's eight
    # k-tiles of f-group fg contiguously, so one DMA loads a whole f-group
    # with 16KB lines (128 descriptors for 2MB).
    w1 = nc.declare_dram_parameter("w1", [FG * 128, KD * 1024], F16, isOutput=False)
    w2 = nc.declare_dram_parameter("w2", [DFF, D], F16, isOutput=False)
    b1 = nc.declare_dram_parameter("b1", [128, FC], F32, isOutput=False)
    b2 = nc.declare_dram_parameter("b2", [128, DM], F32, isOutput=False)
    yT = nc.declare_dram_parameter("yT", [D, cap], F32, isOutput=True)

    gelu = mybir.ActivationFunctionType.Gelu_apprx_tanh

    with ExitStack() as ctx:
        tc = ctx.enter_context(tile.TileContext(nc))
        bpool = ctx.enter_context(tc.tile_pool(name="bias", bufs=1))
        xpool = ctx.enter_context(tc.tile_pool(name="xT", bufs=KD))
        hpool = ctx.enter_context(tc.tile_pool(name="h", bufs=FC))
        w1pool = ctx.enter_context(tc.tile_pool(name="w1", bufs=8))
        wbig_bufs = 2 if cap <= 1024 else 1
        w2pool = ctx.enter_context(tc.tile_pool(name="w2", bufs=4))
        ypool = ctx.enter_context(tc.tile_pool(name="y", bufs=4))

        # First wave: fg0's w1 tiles interleaved with the x tiles (each a
        # single contiguous [128, N] DMA -> ~0.6us trigger each on SP).
        xts = []
        w1t0 = []
        for k in range(KD):
            w = w1pool.tile([128, 1024], F16, name="w0", tag="w0")
            nc.sync.dma_start(w[:], w1[0:128, k * 1024 : (k + 1) * 1024])
            w1t0.append(w)
            xt = xpool.tile([128, cap], F16)
            # scalar-engine HWDGE: x triggers issue in parallel with SP's w1
            # triggers, halving the startup fill.
            nc.scalar.dma_start(xt[:], xT[k * 128 : (k + 1) * 128, :])
            xts.append(xt)

        # Biases are tiny and first needed ~16us in; load via scalar HWDGE
        # to keep the SP trigger queue clear for weights.
        b1_sb = bpool.tile([128, FC], F32, tag="b1")
        nc.scalar.dma_start(b1_sb[:], b1[:, :])
        b2_sb = bpool.tile([128, DM], F32, tag="b2")
        nc.scalar.dma_start(b2_sb[:], b2[:, :])

        # Tokens are processed in blocks of <=2 chunks: h and PSUM footprints
        # stay bounded for arbitrarily skewed dispatch; weights are
        # re-streamed per block (only one block in the common case).
        FB = max(1, 8 // dm_per_pass)  # f-blocks batched per w2 DMA
        w2p = w2.rearrange("(q p) d -> q p d", p=128)
        blocks = []
        c0 = 0
        while c0 < n_chunks:
            blocks.append((c0, min(2, n_chunks - c0)))
            c0 += 2

        for bi, (cb, ncb) in enumerate(blocks):
            tok0 = cb * chunk
            bcap = ncb * chunk

            # ---- phase 1: h = gelu(x @ W1 + b1), feature-major ----
            hs = []
            with tc.tile_pool(name=f"p1_{bi}", bufs=3, space="PSUM") as p1pool:
                for fg in range(FG):
                    if bi == 0 and fg == 0:
                        wbig = None
                    else:
                        wbig = w1pool.tile(
                            [128, KD * 1024],
                            F16,
                            name="wbig",
                            tag="wbig",
                            bufs=wbig_bufs,
                        )
                        nc.sync.dma_start(wbig[:], w1[fg * 128 : (fg + 1) * 128, :])

                    for fl in range(8):
                        f = fg * 8 + fl
                        h = hpool.tile([128, 2 * chunk], F16, name="h", tag="h")
                        for c in range(ncb):
                            t0 = tok0 + c * chunk
                            ps = p1pool.tile([128, chunk], F32, name="ps", tag="ps")
                            for k in range(KD):
                                if wbig is None:
                                    lhsT = w1t0[k][:, fl * 128 : (fl + 1) * 128]
                                else:
                                    lo = k * 1024 + fl * 128
                                    lhsT = wbig[:, lo : lo + 128]
                                nc.tensor.matmul(
                                    ps[:, :],
                                    lhsT,
                                    xts[k][:, t0 : t0 + chunk],
                                    start=(k == 0),
                                    stop=(k == KD - 1),
                                )
                            nc.scalar.activation(
                                h[:, c * chunk : (c + 1) * chunk],
                                ps[:, :],
                                gelu,
                                bias=b1_sb[:, f : f + 1],
                                scale=1.0,
                            )
                        hs.append(h)

            # ---- phase 2: yT = W2 @ h + b2 ----
            # Passes cover dm_per_pass output chunks each and alternate
            # between two PSUM bank sets so pass N+1's matmuls overlap pass
            # N's copy-out.  The host pre-packs w2 so each DMA is one
            # contiguous [128, 1024] slab; triggers go via the scalar
            # engine's HWDGE queue (idle during phase 2).
            with tc.tile_pool(name=f"p2_{bi}", bufs=1, space="PSUM") as p2pool:
                for pz in range(n_pass):
                    dm_lo = pz * dm_per_pass
                    dm_n = min(dm_per_pass, DM - dm_lo)
                    yps = {}
                    for dl in range(dm_n):
                        for c in range(ncb):
                            yps[(dl, c)] = p2pool.tile(
                                [128, chunk],
                                F32,
                                name=f"yp{pz % 2}_{dl}_{c}",
                                tag=f"yp{pz % 2}_{dl}_{c}",
                            )
                    for fq in range(FC // FB):
                        w2t = w2pool.tile([128, FB * dm_per_pass * 128], F16)
                        nc.scalar.dma_start(w2t[:], w2p[pz * (FC // FB) + fq])
                        for fb in range(FB):
                            f = fq * FB + fb
                            for dl in range(dm_n):
                                for c in range(ncb):
                                    nc.tensor.matmul(
                                        yps[(dl, c)][:, :],
                                        w2t[
                                            :,
                                            (fb * dm_n + dl) * 128 : (fb * dm_n + dl + 1) * 128,
                                        ],
                                        hs[f][:, c * chunk : (c + 1) * chunk],
                                        start=(f == 0),
                                        stop=(f == FC - 1),
                                    )
                    for dl in range(dm_n):
                        dm = dm_lo + dl
                        yt = ypool.tile([128, 2 * chunk], F32, name="yt", tag="yt")
                        for c in range(ncb):
                            nc.vector.tensor_scalar_add(
                                yt[:, c * chunk : (c + 1) * chunk],
                                yps[(dl, c)][:, :],
                                b2_sb[:, dm : dm + 1],
                            )
                        nc.sync.dma_start(
                            yT[dm * 128 : (dm + 1) * 128, tok0 : tok0 + bcap],
                            yt[:, :bcap],
                        )

    _split_excess_sync_waits(nc)
    return nc


def _pack_w1(w1e):
    """Pack one expert's w1 fg-major: row fg*128+p holds the partition's
    eight k-tiles of f-group fg contiguously (one 16KB-line DMA per fg)."""
    w = w1e.reshape(KD, 128, FG, 1024)
    packed = w.transpose(2, 1, 0, 3)  # [fg, p, k, 1024]
    return np.ascontiguousarray(packed.reshape(FG * 128, KD * 1024)).astype(
        np.float16
    )


def _pack_w2(w2e, dm_per_pass, n_pass):
    """Pre-pack one expert's w2 so each phase-2 DMA reads one contiguous
    [128, FB*dm_per_pass*128] slab in kernel consumption order."""
    FB = max(1, 8 // dm_per_pass)
    FCB = FC // FB
    w = w2e.reshape(FCB, FB, 128, n_pass, dm_per_pass, 128)
    packed = w.transpose(3, 0, 2, 1, 4, 5)  # [pz, fq, p, fb, dl, d2]
    return np.ascontiguousarray(packed.reshape(DFF, D)).astype(np.float16)


def _enable_trace_hooks():
    """Register the NTFF profile hook (missing antenv.axon_hooks shim)."""
    import types

    if "antenv.axon_hooks" not in sys.modules:
        mod = types.ModuleType("antenv.axon_hooks")
        mod._hook = None

        def set_axon_ntff_profile_hook(h):
            mod._hook = h

        def get_axon_ntff_profile_hook():
            return mod._hook

        mod.set_axon_ntff_profile_hook = set_axon_ntff_profile_hook
        mod.get_axon_ntff_profile_hook = get_axon_ntff_profile_hook
        sys.modules["antenv.axon_hooks"] = mod
        import antenv

        antenv.axon_hooks = mod
    import antenv.axon_hooks as ah

    if ah.get_axon_ntff_profile_hook() is None:
        from trn_agent_boot.trn_boot import _ntff_profile_via_ctypes

        ah.set_axon_ntff_profile_hook(
            _ntff_profile_via_ctypes("/opt/axon/libaxon_pjrt.so")
        )
    import concourse.bass_utils as bu

    bu.upload_artifacts = lambda tmpdir: "local://skipped"


def kernel(inputs, w1, b1, w2, b2, dispatch_order):
    global LAST_EXEC_NS, LAST_RESULT

    inputs = np.asarray(inputs, dtype=np.float32)
    w1 = np.asarray(w1, dtype=np.float32)
    b1 = np.asarray(b1, dtype=np.float32)
    w2 = np.asarray(w2, dtype=np.float32)
    b2 = np.asarray(b2, dtype=np.float32)
    disp = np.asarray(dispatch_order).astype(np.int64)

    B, S, _ = inputs.shape
    T = B * S
    x = inputs.reshape(T, D)

    order = np.argsort(disp, kind="stable")
    counts = np.bincount(disp, minlength=NUM_EXPERTS)
    starts = np.zeros(NUM_EXPERTS + 1, dtype=np.int64)
    np.cumsum(counts, out=starts[1:])

    cap, chunk, n_chunks, dm_per_pass, n_pass = _plan(int(counts.max()))

    key = (cap, chunk, n_chunks, dm_per_pass, n_pass)
    if key not in _NC_CACHE:
        _NC_CACHE[key] = _build_nc(*key)
    nc = _NC_CACHE[key]

    in_maps = []
    for e in range(NUM_EXPERTS):
        toks = order[starts[e] : starts[e + 1]]
        xT_e = np.zeros((D, cap), dtype=np.float16)
        if len(toks):
            xT_e[:, : len(toks)] = x[toks].T
        in_maps.append(
            {
                "xT": xT_e,
                "w1": _pack_w1(w1[e]),
                "w2": _pack_w2(w2[e], dm_per_pass, n_pass),
                "b1": np.ascontiguousarray(b1[e].reshape(FC, 128).T),
                "b2": np.ascontiguousarray(b2[e].reshape(DM, 128).T),
            }
        )

    trace = os.environ.get("MOE_TRACE") == "1"
    kwargs = {}
    if trace:
        _enable_trace_hooks()
        kwargs["trace"] = True
        tmpdir = os.environ.get("MOE_TRACE_DIR")
        if tmpdir:
            os.makedirs(tmpdir, exist_ok=True)
            kwargs["tmpdir"] = tmpdir

    res = run_bass_kernel_spmd(nc, in_maps, list(range(N_CORES)), **kwargs)
    LAST_RESULT = res
    LAST_EXEC_NS = res.exec_time_ns

    out = np.empty((T, D), dtype=np.float32)
    for e in range(NUM_EXPERTS):
        toks = order[starts[e] : starts[e + 1]]
        if len(toks):
            out[toks] = res.results[e]["yT"][:, : len(toks)].T
    return out.reshape(B, S, D)


# revision 37
# speedup vs baseline: 1.0352x; 1.0059x over previous
"""MoE expert-parallel kernel for Trainium2 (8 NeuronCores).

Strategy (expert-parallel, host-side dispatch):
  - Host sorts the T=4096 tokens by dispatch_order. Core e receives the
    tokens routed to expert e, padded to a common capacity Cap, already
    transposed to feature-major xT [D, Cap] (so tokens are always the
    matmul moving/free dimension on device; both weight matrices are used
    in their native layout as the stationary operand).
  - Device (per core): h = gelu_tanh(W1.T-block @ xT + b1) computed
    feature-major [DFF, Cap] in SBUF, then yT = W2-block @ h + b2,
    DMA'd back as [D, Cap]. Matmuls run as float32r (FP22 mantissa
    truncation, full PE rate for free-dim >= 256).
  - Host scatters yT columns back to the original token order.

Self-contained: hardcodes all shapes from the problem spec.
"""

import os
import sys
from contextlib import ExitStack

import numpy as np

for _p in ("/opt/trn_rl_repo",):
    if _p not in sys.path:
        sys.path.insert(0, _p)

import concourse.bass as bass  # noqa: E402
import concourse.tile as tile  # noqa: E402
from concourse import mybir  # noqa: E402
from concourse.bass_utils import run_bass_kernel_spmd  # noqa: E402

# ---------------------------------------------------------------------------
# Workaround for this walrus build: a Drain instruction with >1 sem wait
# fails codegen ("Too many sync wait commands").  Replace the Tile
# kernel-tail drain with single-wait SP nops followed by a bare drain.
# ---------------------------------------------------------------------------


def _patched_drain_and_barrier(self, tick_clock, wait_clock):
    from concourse.vector_clock import ScopedClock

    nc = self.nc
    probe = nc.sync.nop(nofuse=True)
    wait_clock.add_sem_waits(probe.ins, ScopedClock({None: tick_clock.global_clock}))
    si = probe.ins.sync_info
    waits = list(si.on_wait) if si and si.on_wait else []
    probe.ins.sync_info = mybir.SyncInfo(on_wait=waits[:1], on_update=[])
    for w in waits[1:]:
        n = nc.sync.nop(nofuse=True)
        n.ins.sync_info = mybir.SyncInfo(on_wait=[w], on_update=[])

    nc.sync.drain()
    nc.all_engine_barrier()
    assert self.sems is not None
    popped = nc._tile_sem_poison_stack.pop()
    assert popped is self._sem_poison
    nc.clear_and_free_semaphores(list(self.sems.allocated().values()))
    nc.all_engine_barrier()


tile.TileContext._drain_and_barrier = _patched_drain_and_barrier


def _split_excess_sync_waits(nc, max_waits=1):
    """This walrus build only encodes one sem wait per instruction.  Hoist
    excess waits onto same-engine nops inserted immediately before."""
    for f in nc.m.functions:
        for bb in f.blocks:
            out = []
            for inst in bb.instructions:
                si = inst.sync_info
                if si and si.on_wait and len(si.on_wait) > max_waits:
                    waits = list(si.on_wait)
                    for i in range(max_waits, len(waits), max_waits):
                        n = mybir.InstNoOp(
                            name=f"{inst.name}-waitsplit-{i}", ins=[], outs=[]
                        )
                        n.engine = inst.engine
                        n.sync_info = mybir.SyncInfo(
                            on_wait=waits[i : i + max_waits], on_update=[]
                        )
                        out.append(n)
                    inst.sync_info = mybir.SyncInfo(
                        on_wait=waits[:max_waits], on_update=list(si.on_update or [])
                    )
                out.append(inst)
            bb.instructions[:] = out


# ---------------------------------------------------------------------------

NUM_EXPERTS = 8
D = 1024
DFF = 4096
N_CORES = 8
KD = D // 128  # 8 k-chunks for matmul 1
FC = DFF // 128  # 32 f-chunks
DM = D // 128  # 8 output chunks
FG = 4  # f-groups for w1 streaming (each 8 f-chunks = 1024 cols)

F32 = mybir.dt.float32
F32R = mybir.dt.float32r
F16 = mybir.dt.float16

LAST_EXEC_NS = None
LAST_RESULT = None

_NC_CACHE = {}


def _plan(max_count):
    """Pick capacity/chunking: equal token chunks, each in [256, 512].

    Chunks >= ~256 keep the per-matmul LDWEIGHTS (~100 ns) hidden behind
    the matmul stream (chunk/2.4GHz ns)."""
    n_chunks = max(1, -(-max_count // 512))
    chunk = -(-max_count // n_chunks)
    chunk = max(256, -(-chunk // 8) * 8)
    cap = chunk * n_chunks
    # Tokens are processed in blocks of <=2 chunks so h/PSUM stay bounded
    # for arbitrarily skewed dispatch.  Phase 2: each pass uses
    # dm_per_pass*block_chunks PSUM banks (<=4), so two pass-sets alternate
    # within the 8 banks and passes overlap.  The last uniform pass is
    # split into single-dm passes so the kernel tail's copy-out is minimal.
    dm_per_pass = max(1, 4 // min(n_chunks, 2))
    n_pass = -(-DM // dm_per_pass)
    return cap, chunk, n_chunks, dm_per_pass, n_pass


def _dm_schedule(dm_per_pass):
    sched = [dm_per_pass] * (DM // dm_per_pass)
    sched[-1:] = {1: [1], 2: [1, 1], 4: [2, 1, 1]}[sched[-1]]
    assert sum(sched) == DM
    return sched


def _build_nc(cap, chunk, n_chunks, dm_per_pass, n_pass):
    nc = bass.Bass()
    xT = nc.declare_dram_parameter("xT", [D, cap], F16, isOutput=False)
    # w1 is host-packed fg-major: row fg*128+p holds that partition's eight
    # k-tiles of f-group fg contiguously, so one DMA loads a whole f-group
    # with 16KB lines (128 descriptors for 2MB).
    w1 = nc.declare_dram_parameter("w1", [FG * 128, KD * 1024], F16, isOutput=False)
    w2 = nc.declare_dram_parameter("w2", [DFF, D], F16, isOutput=False)
    b1 = nc.declare_dram_parameter("b1", [128, FC], F32, isOutput=False)
    b2 = nc.declare_dram_parameter("b2", [128, DM], F32, isOutput=False)
    yT = nc.declare_dram_parameter("yT", [D, cap], F32, isOutput=True)

    gelu = mybir.ActivationFunctionType.Gelu_apprx_tanh

    with ExitStack() as ctx:
        tc = ctx.enter_context(tile.TileContext(nc))
        bpool = ctx.enter_context(tc.tile_pool(name="bias", bufs=1))
        xpool = ctx.enter_context(tc.tile_pool(name="xT", bufs=KD))
        hpool = ctx.enter_context(tc.tile_pool(name="h", bufs=FC))
        w1pool = ctx.enter_context(tc.tile_pool(name="w1", bufs=8))
        wbig_bufs = 2 if cap <= 1024 else 1
        w2pool = ctx.enter_context(tc.tile_pool(name="w2", bufs=4))
        ypool = ctx.enter_context(tc.tile_pool(name="y", bufs=4))

        # First wave: fg0's w1 tiles interleaved with the x tiles (each a
        # single contiguous [128, N] DMA -> ~0.6us trigger each on SP).
        xts = []
        w1t0 = []
        for k in range(KD):
            w = w1pool.tile([128, 1024], F16, name="w0", tag="w0")
            nc.sync.dma_start(w[:], w1[0:128, k * 1024 : (k + 1) * 1024])
            w1t0.append(w)
            xt = xpool.tile([128, cap], F16)
            # scalar-engine HWDGE: x triggers issue in parallel with SP's w1
            # triggers, halving the startup fill.
            nc.scalar.dma_start(xt[:], xT[k * 128 : (k + 1) * 128, :])
            xts.append(xt)

        # Biases are tiny and first needed ~16us in; load via scalar HWDGE
        # to keep the SP trigger queue clear for weights.
        b1_sb = bpool.tile([128, FC], F32, tag="b1")
        nc.scalar.dma_start(b1_sb[:], b1[:, :])
        b2_sb = bpool.tile([128, DM], F32, tag="b2")
        nc.scalar.dma_start(b2_sb[:], b2[:, :])

        # Tokens are processed in blocks of <=2 chunks: h and PSUM footprints
        # stay bounded for arbitrarily skewed dispatch; weights are
        # re-streamed per block (only one block in the common case).
        FB = max(1, 8 // dm_per_pass)  # f-blocks batched per w2 DMA
        w2p = w2.rearrange("(q p) d -> q p d", p=128)
        blocks = []
        c0 = 0
        while c0 < n_chunks:
            blocks.append((c0, min(2, n_chunks - c0)))
            c0 += 2

        for bi, (cb, ncb) in enumerate(blocks):
            tok0 = cb * chunk
            bcap = ncb * chunk

            # ---- phase 1: h = gelu(x @ W1 + b1), feature-major ----
            hs = []
            with tc.tile_pool(name=f"p1_{bi}", bufs=3, space="PSUM") as p1pool:
                for fg in range(FG):
                    if bi == 0 and fg == 0:
                        wbig = None
                    else:
                        wbig = w1pool.tile(
                            [128, KD * 1024],
                            F16,
                            name="wbig",
                            tag="wbig",
                            bufs=wbig_bufs,
                        )
                        nc.sync.dma_start(wbig[:], w1[fg * 128 : (fg + 1) * 128, :])

                    for fl in range(8):
                        f = fg * 8 + fl
                        h = hpool.tile([128, 2 * chunk], F16, name="h", tag="h")
                        for c in range(ncb):
                            t0 = tok0 + c * chunk
                            ps = p1pool.tile([128, chunk], F32, name="ps", tag="ps")
                            for k in range(KD):
                                if wbig is None:
                                    lhsT = w1t0[k][:, fl * 128 : (fl + 1) * 128]
                                else:
                                    lo = k * 1024 + fl * 128
                                    lhsT = wbig[:, lo : lo + 128]
                                nc.tensor.matmul(
                                    ps[:, :],
                                    lhsT,
                                    xts[k][:, t0 : t0 + chunk],
                                    start=(k == 0),
                                    stop=(k == KD - 1),
                                )
                            nc.scalar.activation(
                                h[:, c * chunk : (c + 1) * chunk],
                                ps[:, :],
                                gelu,
                                bias=b1_sb[:, f : f + 1],
                                scale=1.0,
                            )
                        hs.append(h)

            # ---- phase 2: yT = W2 @ h + b2 ----
            # Passes cover dm_per_pass output chunks each and alternate
            # between two PSUM bank sets so pass N+1's matmuls overlap pass
            # N's copy-out.  The host pre-packs w2 so each DMA is one
            # contiguous [128, 1024] slab; triggers go via the scalar
            # engine's HWDGE queue (idle during phase 2).
            with tc.tile_pool(name=f"p2_{bi}", bufs=1, space="PSUM") as p2pool:
                sched = _dm_schedule(dm_per_pass)
                qrow = 0  # row-block cursor into the packed w2
                dm_lo = 0
                for pz, dm_n in enumerate(sched):
                    fbn = 8 // dm_n  # f-blocks per 1024-col w2 slab
                    yps = {}
                    for dl in range(dm_n):
                        for c in range(ncb):
                            yps[(dl, c)] = p2pool.tile(
                                [128, chunk],
                                F32,
                                name=f"yp{pz % 2}_{dl}_{c}",
                                tag=f"yp{pz % 2}_{dl}_{c}",
                            )
                    for fq in range(FC // fbn):
                        w2t = w2pool.tile([128, 1024], F16, name="w2t", tag="w2t")
                        nc.scalar.dma_start(w2t[:], w2p[qrow])
                        qrow += 1
                        for fb in range(fbn):
                            f = fq * fbn + fb
                            for dl in range(dm_n):
                                for c in range(ncb):
                                    nc.tensor.matmul(
                                        yps[(dl, c)][:, :],
                                        w2t[
                                            :,
                                            (fb * dm_n + dl) * 128 : (fb * dm_n + dl + 1) * 128,
                                        ],
                                        hs[f][:, c * chunk : (c + 1) * chunk],
                                        start=(f == 0),
                                        stop=(f == FC - 1),
                                    )
                    for dl in range(dm_n):
                        dm = dm_lo + dl
                        yt = ypool.tile([128, 2 * chunk], F32, name="yt", tag="yt")
                        for c in range(ncb):
                            nc.vector.tensor_scalar_add(
                                yt[:, c * chunk : (c + 1) * chunk],
                                yps[(dl, c)][:, :],
                                b2_sb[:, dm : dm + 1],
                            )
                            nc.sync.dma_start(
                                yT[
                                    dm * 128 : (dm + 1) * 128,
                                    tok0 + c * chunk : tok0 + (c + 1) * chunk,
                                ],
                                yt[:, c * chunk : (c + 1) * chunk],
                            )
                    dm_lo += dm_n

    _split_excess_sync_waits(nc)
    return nc


def _pack_w1(w1e):
    """Pack one expert's w1 fg-major: row fg*128+p holds the partition's
    eight k-tiles of f-group fg contiguously (one 16KB-line DMA per fg)."""
    w = w1e.reshape(KD, 128, FG, 1024)
    packed = w.transpose(2, 1, 0, 3)  # [fg, p, k, 1024]
    return np.ascontiguousarray(packed.reshape(FG * 128, KD * 1024)).astype(
        np.float16
    )


def _pack_w2(w2e, dm_per_pass, n_pass):
    """Pre-pack one expert's w2 into [128, 1024] slabs in exact kernel
    consumption order (following the phase-2 dm pass schedule)."""
    sched = _dm_schedule(dm_per_pass)
    w = w2e.reshape(FC, 128, DM, 128)  # [f, p, dm, d2]
    slabs = []
    dm_lo = 0
    for dm_n in sched:
        fbn = 8 // dm_n
        for fq in range(FC // fbn):
            slab = np.empty((128, 1024), np.float32)
            for fb in range(fbn):
                f = fq * fbn + fb
                for dl in range(dm_n):
                    lo = (fb * dm_n + dl) * 128
                    slab[:, lo : lo + 128] = w[f, :, dm_lo + dl, :]
            slabs.append(slab)
        dm_lo += dm_n
    return np.concatenate(slabs, axis=0).astype(np.float16)


def _enable_trace_hooks():
    """Register the NTFF profile hook (missing antenv.axon_hooks shim)."""
    import types

    if "antenv.axon_hooks" not in sys.modules:
        mod = types.ModuleType("antenv.axon_hooks")
        mod._hook = None

        def set_axon_ntff_profile_hook(h):
            mod._hook = h

        def get_axon_ntff_profile_hook():
            return mod._hook

        mod.set_axon_ntff_profile_hook = set_axon_ntff_profile_hook
        mod.get_axon_ntff_profile_hook = get_axon_ntff_profile_hook
        sys.modules["antenv.axon_hooks"] = mod
        import antenv

        antenv.axon_hooks = mod
    import antenv.axon_hooks as ah

    if ah.get_axon_ntff_profile_hook() is None:
        from trn_agent_boot.trn_boot import _ntff_profile_via_ctypes

        ah.set_axon_ntff_profile_hook(
            _ntff_profile_via_ctypes("/opt/axon/libaxon_pjrt.so")
        )
    import concourse.bass_utils as bu

    bu.upload_artifacts = lambda tmpdir: "local://skipped"


def kernel(inputs, w1, b1, w2, b2, dispatch_order):
    global LAST_EXEC_NS, LAST_RESULT

    inputs = np.asarray(inputs, dtype=np.float32)
    w1 = np.asarray(w1, dtype=np.float32)
    b1 = np.asarray(b1, dtype=np.float32)
    w2 = np.asarray(w2, dtype=np.float32)
    b2 = np.asarray(b2, dtype=np.float32)
    disp = np.asarray(dispatch_order).astype(np.int64)

    B, S, _ = inputs.shape
    T = B * S
    x = inputs.reshape(T, D)

    order = np.argsort(disp, kind="stable")
    counts = np.bincount(disp, minlength=NUM_EXPERTS)
    starts = np.zeros(NUM_EXPERTS + 1, dtype=np.int64)
    np.cumsum(counts, out=starts[1:])

    cap, chunk, n_chunks, dm_per_pass, n_pass = _plan(int(counts.max()))

    key = (cap, chunk, n_chunks, dm_per_pass, n_pass)
    if key not in _NC_CACHE:
        _NC_CACHE[key] = _build_nc(*key)
    nc = _NC_CACHE[key]

    in_maps = []
    for e in range(NUM_EXPERTS):
        toks = order[starts[e] : starts[e + 1]]
        xT_e = np.zeros((D, cap), dtype=np.float16)
        if len(toks):
            xT_e[:, : len(toks)] = x[toks].T
        in_maps.append(
            {
                "xT": xT_e,
                "w1": _pack_w1(w1[e]),
                "w2": _pack_w2(w2[e], dm_per_pass, n_pass),
                "b1": np.ascontiguousarray(b1[e].reshape(FC, 128).T),
                "b2": np.ascontiguousarray(b2[e].reshape(DM, 128).T),
            }
        )

    trace = os.environ.get("MOE_TRACE") == "1"
    kwargs = {}
    if trace:
        _enable_trace_hooks()
        kwargs["trace"] = True
        tmpdir = os.environ.get("MOE_TRACE_DIR")
        if tmpdir:
            os.makedirs(tmpdir, exist_ok=True)
            kwargs["tmpdir"] = tmpdir

    res = run_bass_kernel_spmd(nc, in_maps, list(range(N_CORES)), **kwargs)
    LAST_RESULT = res
    LAST_EXEC_NS = res.exec_time_ns

    out = np.empty((T, D), dtype=np.float32)
    for e in range(NUM_EXPERTS):
        toks = order[starts[e] : starts[e + 1]]
        if len(toks):
            out[toks] = res.results[e]["yT"][:, : len(toks)].T
    return out.reshape(B, S, D)
